# revision 1
# baseline (speedup 1.0000x reference)
"""Trainium2 Bass kernel for multi-head attention (B=4, N=2048, C=512, 8 heads).

Sharding: 8 cores = (batch b = core//2) x (head-group g = core%2, 4 heads each).
Per core, a transposed-scores attention pipeline:
  - host supplies x[b] transposed (xT [C, N]) and per-group transposed weights,
    all pre-cast to fp16 (matmul streams at 1 cycle/row; ~4x the mantissa of
    bf16; every tensor here fits fp16 range comfortably)
  - qT/kT stored zero-padded per head ([:, hh, :] has head hh's 64 dims on
    its own partition range, rest zero) so score matmuls contract over the
    full K=128 partition range: same N cycles as K=64, but the PE activity
    monitor sees a fully-active array and keeps the 2.4 GHz clock (K=64
    matmuls measure at the 1.2 GHz throttled rate)
  - v as [N, (1+64) per head] tiles; the leading ones column makes attn@v
    emit the softmax denominator into PSUM partition 0
  - 8 single-head sections of 16 ktok blocks; scores^T [128, 1024] in PSUM,
    exp on ACT (PSUM -> SBUF fp16), attn@v accumulated in PSUM; attn@v is
    emitted one block behind scores (the PE is in-order - it must never sit
    at an attn@v waiting on a just-issued exp)
  - phase-A work that section 0 does not need trickles in as per-block
    "fillers" so the ACT engine starts its exp stream as early as possible
  - normalization entirely off the PE: DVE fast-reciprocal (partition 0),
    GpSimd partition_broadcast, DVE multiply, DMA partition-shift into outT
  - output projection on-device (first half woven into the last section);
    host sums the two half-head partials
"""

import sys

sys.path.insert(0, "/opt/trn_rl_repo")

import numpy as np

B, N, C = 4, 2048, 512
H, D = 8, 64
SCALE = float(D) ** -0.5  # 0.125, exact in fp32
P = 128
CT = C // P  # 4 contraction tiles over channels
NT = N // P  # 16 token blocks
NCORES = 8
FD = 1024  # softmax block free dim (q chunk)
QH = N // FD  # 2 q halves

_cache = {}


def _build():
    import concourse.bacc as bacc
    import concourse.tile as tile
    from concourse import mybir

    f32 = mybir.dt.float32
    f16 = mybir.dt.float16
    u16 = mybir.dt.uint16
    EXP = mybir.ActivationFunctionType.Exp

    nc = bacc.Bacc("TRN2", target_bir_lowering=False, debug=False,
                   num_devices=NCORES)

    xT_d = nc.dram_tensor("xT", [C, N], f16, kind="ExternalInput")
    wqT_d = nc.dram_tensor("wqT", [P, CT * 256], f16, kind="ExternalInput")
    wkT_d = nc.dram_tensor("wkT", [P, CT * 256], f16, kind="ExternalInput")
    wvT_d = nc.dram_tensor("wvT", [P, CT * 256], f16, kind="ExternalInput")
    pwT_d = nc.dram_tensor("pwT", [P, 2 * C], f16, kind="ExternalInput")
    y_d = nc.dram_tensor("y", [N, C], f32, kind="ExternalOutput")

    with tile.TileContext(nc) as tc:
        with (
            tc.tile_pool(name="io", bufs=1) as io,
            tc.tile_pool(name="qk", bufs=1) as qk,
            tc.tile_pool(name="expp", bufs=6) as expp,
            tc.tile_pool(name="workp", bufs=3) as workp,
            tc.tile_pool(name="yp", bufs=4) as yp,
            tc.tile_pool(name="ps_s", bufs=2, space="PSUM") as ps_s,
            tc.tile_pool(name="ps_o", bufs=2, space="PSUM") as ps_o,
        ):
            # ---- input loads (fine-grained: first matmul starts early) ----
            xT_sb = io.tile([P, CT, N], f16, tag="xT", name="xT_sb")
            xT_ap = xT_d[:].rearrange("(t p) n -> p t n", p=P)
            for t in range(CT):
                nc.sync.dma_start(xT_sb[:, t, 0:1024], xT_ap[:, t, 0:1024])
            for t in range(CT):
                nc.sync.dma_start(xT_sb[:, t, 1024:2048],
                                  xT_ap[:, t, 1024:2048])

            wq_sb = io.tile([P, CT, 256], f16, tag="wq", name="wq_sb")
            nc.sync.dma_start(
                wq_sb[:], wqT_d[:].rearrange("p (t m) -> p t m", t=CT))
            wk_sb = io.tile([P, CT, 256], f16, tag="wk", name="wk_sb")
            nc.sync.dma_start(
                wk_sb[:], wkT_d[:].rearrange("p (t m) -> p t m", t=CT))
            wv_sb = io.tile([P, CT, 256], f16, tag="wv", name="wv_sb")
            nc.sync.dma_start(
                wv_sb[:], wvT_d[:].rearrange("p (t m) -> p t m", t=CT))
            pw_sb = io.tile([P, 2, C], f16, tag="pw", name="pw_sb")
            nc.sync.dma_start(
                pw_sb[:], pwT_d[:].rearrange("p (t m) -> p t m", t=2))

            # ---- SBUF persistents ----
            qT = []
            kT = []
            vv = []
            outT = []
            for p in range(2):
                qT.append(qk.tile([P, 2, N], f16, tag=f"qT{p}", name=f"qT{p}"))
                kT.append(qk.tile([P, 2, N], f16, tag=f"kT{p}", name=f"kT{p}"))
                vv.append(qk.tile([P, NT, 130], f16, tag=f"v{p}", name=f"v{p}"))
                outT.append(qk.tile([P, N], f16, tag=f"outT{p}", name=f"outT{p}"))

            # trigger the ACT exp table load during the DMA ramp
            scratch1 = io.tile([1, 2], f32, tag="scratch1", name="scratch1")
            nc.vector.memset(scratch1[:], 0.0)
            nc.scalar.activation(scratch1[0:1, 0:1], scratch1[0:1, 1:2], EXP)
            # zero padding halves for pair 0 now (GpSimd, off the DMA/DVE
            # ramp); pair 1's pads are deferred into the filler stream
            nc.gpsimd.memset(kT[0][64:128, 0, :], 0.0)
            nc.gpsimd.memset(kT[0][0:64, 1, :], 0.0)
            nc.gpsimd.memset(qT[0][64:128, 0, :], 0.0)
            nc.gpsimd.memset(qT[0][0:64, 1, :], 0.0)
            for p in range(2):
                # ones columns (fp16 1.0) at the head of each v block (DVE:
                # strided 16-element memsets are fast there, slow on GpSimd)
                nc.vector.memset(vv[p][:, :, 0:1].bitcast(u16), 0x3C00)
                nc.vector.memset(vv[p][:, :, 65:66].bitcast(u16), 0x3C00)

            def emit_qk_chunk(p, w_sb, dst, ch, dve_only=False):
                pc = slice(128 * p, 128 * (p + 1))
                cs = slice(512 * ch, 512 * (ch + 1))
                ps = ps_s.tile([P, FD], f32, tag="s",
                               name=f"qkps_{p}_{ch}_{w_sb.tensor.name}")
                for t in range(CT):
                    nc.tensor.matmul(
                        ps[:, :512],
                        lhsT=w_sb[:, t, pc],
                        rhs=xT_sb[:, t, cs],
                        start=(t == 0), stop=(t == CT - 1))
                nc.vector.tensor_copy(dst[0:64, 0, cs], ps[0:64, :512])
                if dve_only:
                    nc.vector.tensor_copy(dst[64:128, 1, cs], ps[64:128, :512])
                else:
                    nc.scalar.copy(dst[64:128, 1, cs], ps[64:128, :512])

            def emit_v_tile(tt):
                psv = ps_s.tile([P, FD], f32, tag="s", name=f"vps_{tt}")
                for t in range(CT):
                    nc.tensor.matmul(
                        psv[:, :256],
                        lhsT=xT_sb[:, t, 128 * tt:128 * (tt + 1)],
                        rhs=wv_sb[:, t, 0:256],
                        start=(t == 0), stop=(t == CT - 1))
                for p in range(2):
                    pv = psv[:, 128 * p:128 * (p + 1)].rearrange(
                        "p (two d) -> p two d", two=2)
                    dv = vv[p][:, tt, 0:130].rearrange(
                        "p (two d65) -> p two d65", two=2)[:, :, 1:65]
                    nc.vector.tensor_copy(dv, pv)

            def emit_y_block(tt, act_evict=True):
                yps = ps_s.tile([P, FD], f32, tag="s", name=f"yps_{tt}")
                for p in range(2):
                    nc.tensor.matmul(
                        yps[:, :512], lhsT=outT[p][:, 128 * tt:128 * (tt + 1)],
                        rhs=pw_sb[:, p, :], start=(p == 0), stop=(p == 1))
                ys = yp.tile([P, C], f32, tag="y", name=f"ys_{tt}")
                if act_evict:
                    nc.scalar.copy(ys[:], yps[:, :512])
                else:
                    nc.vector.tensor_copy(ys[:], yps[:, :512])
                nc.sync.dma_start(y_d[128 * tt:128 * (tt + 1), :], ys[:])

            fillers = []

            def pop_fillers(k):
                for _ in range(k):
                    if fillers:
                        fillers.pop(0)()

            def norm_head(p, hh, qh, o):
                qs = slice(FD * qh, FD * (qh + 1))
                r = workp.tile([P, FD], f32, tag="r", name=f"r_{p}_{hh}_{qh}")
                nc.vector.reciprocal_approx_fast(r[0:1, :], o[0:1, :])
                rb = workp.tile([65, FD], f32, tag="rb",
                                name=f"rb_{p}_{hh}_{qh}")
                nc.gpsimd.partition_broadcast(rb[:], r[0:1, :])
                st = workp.tile([65, FD], f16, tag="st",
                                name=f"st_{p}_{hh}_{qh}")
                nc.vector.tensor_mul(st[:], o[:], rb[:])
                nc.sync.dma_start(outT[p][64 * hh:64 * (hh + 1), qs],
                                  st[1:65, :])

            def emit_section(p, hh, qh, pops=1):
                vs = slice(65 * hh, 65 * (hh + 1))
                o = ps_o.tile([65, FD], f32, tag="o", name=f"o_{p}_{hh}_{qh}")

                def emit_scores_exp(i):
                    ks = slice(128 * i, 128 * (i + 1))
                    s = ps_s.tile([P, FD], f32, tag="s",
                                  name=f"s_{p}_{hh}_{qh}_{i}")
                    for j in range(2):
                        js = slice(512 * j, 512 * (j + 1))
                        qj = slice(FD * qh + 512 * j, FD * qh + 512 * (j + 1))
                        nc.tensor.matmul(
                            s[:, js], lhsT=kT[p][:, hh, ks],
                            rhs=qT[p][:, hh, qj], start=True, stop=True)
                    e = expp.tile([P, FD], f16, tag="exp",
                                  name=f"e_{p}_{hh}_{qh}_{i}")
                    nc.scalar.activation(e[:], s[:], EXP)
                    return e

                def emit_attnv(i, e):
                    for j in range(2):
                        js = slice(512 * j, 512 * (j + 1))
                        nc.tensor.matmul(
                            o[:, js], lhsT=vv[p][:, i, vs], rhs=e[:, js],
                            start=(i == 0), stop=(i == NT - 1))

                pending = None
                for i in range(NT):
                    e = emit_scores_exp(i)
                    if pending is not None:
                        emit_attnv(i - 1, pending)
                    pending = e
                    pop_fillers(pops)
                emit_attnv(NT - 1, pending)

                norm_head(p, hh, qh, o)

            # critical prefix: everything section (0,0,0) touches early
            for ch in range(4):
                emit_qk_chunk(0, wk_sb, kT[0], ch)
            for ch in range(2):
                emit_qk_chunk(0, wq_sb, qT[0], ch)
            for tt in range(4):
                emit_v_tile(tt)
            # the rest of phase A trickles in between section blocks
            fillers.append(lambda: nc.gpsimd.memset(kT[1][64:128, 0, :], 0.0))
            fillers.append(lambda: nc.gpsimd.memset(kT[1][0:64, 1, :], 0.0))
            fillers.append(lambda: nc.gpsimd.memset(qT[1][64:128, 0, :], 0.0))
            fillers.append(lambda: nc.gpsimd.memset(qT[1][0:64, 1, :], 0.0))
            for tt in range(4, NT):
                fillers.append(lambda tt=tt: emit_v_tile(tt))
            for ch in range(2, 4):
                fillers.append(lambda ch=ch: emit_qk_chunk(
                    0, wq_sb, qT[0], ch, dve_only=True))
            for ch in range(4):
                fillers.append(lambda ch=ch: emit_qk_chunk(
                    1, wk_sb, kT[1], ch, dve_only=True))
            for ch in range(4):
                fillers.append(lambda ch=ch: emit_qk_chunk(
                    1, wq_sb, qT[1], ch, dve_only=True))

            sections = [(p, hh, qh) for p in range(2) for hh in range(2)
                        for qh in range(QH)]
            for idx, (p, hh, qh) in enumerate(sections):
                emit_section(p, hh, qh, pops=2 if idx == 0 else 1)
            pop_fillers(len(fillers))

            # ---- phase C: output projection ----
            for tt in range(NT):
                emit_y_block(tt, act_evict=(tt % 2 == 0))

    nc.finalize()
    return nc


def _get_nc():
    if "nc" not in _cache:
        _cache["nc"] = _build()
    return _cache["nc"]


def _pack(wt, groups):
    # [G*128, M] row-major -> [128, G*M]: partition p holds the concat over
    # groups of row (g*128 + p), so the DMA reads one contiguous run per p
    g128, m = wt.shape
    assert g128 == groups * 128
    return np.ascontiguousarray(
        wt.reshape(groups, 128, m).transpose(1, 0, 2).reshape(128, groups * m))


def _make_in_maps(x, q_w, kv_w, proj_w):
    x = np.asarray(x, dtype=np.float32)
    q_w = np.asarray(q_w, dtype=np.float32)
    kv_w = np.asarray(kv_w, dtype=np.float32)
    proj_w = np.asarray(proj_w, dtype=np.float32)
    f16 = np.float16
    in_maps = []
    for core in range(NCORES):
        b, g = core // 2, core % 2
        hs = slice(g * 256, (g + 1) * 256)
        in_maps.append({
            "xT": np.ascontiguousarray(x[b].T.astype(f16)),
            "wqT": _pack((q_w[hs, :] * np.float32(SCALE)).T.astype(f16), CT),
            "wkT": _pack(kv_w[hs, :].T.astype(f16), CT),
            "wvT": _pack(
                kv_w[C + g * 256:C + (g + 1) * 256, :].T.astype(f16), CT),
            "pwT": _pack(proj_w[:, hs].T.astype(f16), 2),
        })
    return in_maps


def kernel(x, q_w, kv_w, proj_w, proj_b, H=None, W=None, _trace=False):
    from concourse.bass_utils import run_bass_kernel_spmd

    nc = _get_nc()
    in_maps = _make_in_maps(x, q_w, kv_w, proj_w)
    res = run_bass_kernel_spmd(nc, in_maps, core_ids=list(range(NCORES)),
                               trace=_trace)
    proj_b = np.asarray(proj_b, dtype=np.float32)
    out = np.empty((B, N, C), dtype=np.float32)
    for b in range(B):
        out[b] = res.results[2 * b]["y"] + res.results[2 * b + 1]["y"] + proj_b
    if _trace:
        return out, res
    return out



# revision 2
# speedup vs baseline: 1.2353x; 1.2353x over previous
"""Trainium2 Bass kernel for multi-head attention (B=4, N=2048, C=512, 8 heads).

Sharding: 8 cores = (batch b = core//2) x (head-group g = core%2, 4 heads each).

v2 pipeline (per core, 4 heads as 2 pairs):
  - q/k stored PACKED fp16: qT[p]/kT[p] [128 = headA d0-63 | headB d0-63, N].
    No zero padding anywhere: scores for the two heads of a pair run as two
    CONCURRENT row-tiled K=64 matmuls (tile_position (0,0) / (64,0)) writing
    the two halves of one [128, 1024] PSUM tile -> the pair's score block
    costs one matmul span (~220ns) instead of two.
  - one exp per block over the paired [128, 1024] PSUM tile. Exp work is
    split between ACT (nc.scalar.activation Exp) and a custom 8-stage DVE
    op (EXP_POLY3SQ2_ANT: cubic in x then two squarings ~ exp(x), max rel
    err 1.9e-3 on |logit| <= 2.05) so the two engines share the softmax.
  - v as [N, (1+64) per head] tiles; leading ones column emits the softmax
    denominator into PSUM row 0 of each head's [65, 512] accumulator.
  - sections = (pair p, q-chunk of 512) x 16 k-blocks; attnv trails scores
    by one block (in-order PE must never wait on a just-issued exp).
  - phase-A work (qkv projections, v tiles) and phase-C y-blocks trickle in
    as per-block fillers so ACT/DVE start early and PE never idles long
    (keeps the HAM clock gate at 2.4 GHz).
  - normalization off the PE: DVE fast-reciprocal, GpSimd partition
    broadcast, DVE multiply, DMA partition-shift into outT.
  - output projection on-device; host sums the two half-head partials.
"""

import sys

sys.path.insert(0, "/opt/trn_rl_repo")

import numpy as np

B, N, C = 4, 2048, 512
H, D = 8, 64
SCALE = float(D) ** -0.5  # 0.125, exact
P = 128
CT = C // P   # 4 contraction tiles over channels
NT = N // P   # 16 token blocks
QC = 4        # q chunks of 512
NCORES = 8

# custom DVE exp: p(x) = ((c3*x + c2)*x + c1)*x + 1, exp(x) ~ p(x)^4
EC3, EC2, EC1 = 0.0025544826062447396, 0.03181193776331223, 0.2502295107773785

_cache = {}


def _register_exp_op():
    import concourse.dve_ops as dve_ops
    from concourse.dve_ops import DveOp
    from concourse.dve_spec import C0 as _C0
    from concourse.dve_spec import C1 as _C1
    from concourse.dve_spec import C2 as _C2
    from concourse.dve_spec import One, Spec, Src0, lower, sq
    from concourse.dve_uop import DveOpSpec

    name = "EXP_POLY3SQ2_ANT"
    for op in dve_ops.OPS:
        if op.name == name:
            return op

    body = sq(sq(((Src0 * _C0 + _C1) * Src0 + _C2) * Src0 + One))

    def ref(in0, in1, c0, c1, c2):
        p = ((in0 * c0 + c1) * in0 + c2) * in0 + 1.0
        p = p * p
        return (p * p).astype(np.float32)

    spec = Spec(body=body, reference=ref)
    row = dve_ops._CUSTOM_DVE_ROW_BASE + len(dve_ops.OPS)
    assert row < 0x20
    dve_ops._SUB_OPCODE_FOR_NAME[name] = row
    shas = {}
    for ver in ("v3", "v4"):
        d = DveOpSpec(name=name, opcode=row, uops=lower(spec, ver=ver),
                      rd1_en=False)
        shas[ver] = d.sha(ver)
    op = DveOp(name, spec, subdim=False, uops_sha=shas)
    dve_ops.OPS.append(op)
    dve_ops.CUSTOM_DVE_SPECS[name] = spec
    return op


def _build():
    import concourse.bacc as bacc
    import concourse.tile as tile
    from concourse import mybir

    f32 = mybir.dt.float32
    f16 = mybir.dt.float16
    u16 = mybir.dt.uint16
    EXP = mybir.ActivationFunctionType.Exp

    exp_op = _register_exp_op()

    nc = bacc.Bacc("TRN2", target_bir_lowering=False, debug=False,
                   num_devices=NCORES)

    xT_d = nc.dram_tensor("xT", [C, N], f16, kind="ExternalInput")
    wqT_d = nc.dram_tensor("wqT", [P, CT * 256], f16, kind="ExternalInput")
    wkT_d = nc.dram_tensor("wkT", [P, CT * 256], f16, kind="ExternalInput")
    wvT_d = nc.dram_tensor("wvT", [P, CT * 256], f16, kind="ExternalInput")
    pwT_d = nc.dram_tensor("pwT", [P, 2 * C], f16, kind="ExternalInput")
    y_d = nc.dram_tensor("y", [N, C], f32, kind="ExternalOutput")

    with tile.TileContext(nc) as tc:
        with (
            tc.tile_pool(name="io", bufs=1) as io,
            tc.tile_pool(name="qk", bufs=1) as qk,
            tc.tile_pool(name="expp", bufs=6) as expp,
            tc.tile_pool(name="workp", bufs=4) as workp,
            tc.tile_pool(name="yp", bufs=4) as yp,
            tc.tile_pool(name="ps_s", bufs=3, space="PSUM") as ps_s,
            tc.tile_pool(name="ps_o", bufs=2, space="PSUM") as ps_o,
        ):
            # ---- input loads (fine-grained: first matmul starts early) ----
            xT_sb = io.tile([P, CT, N], f16, tag="xT", name="xT_sb")
            xT_ap = xT_d[:].rearrange("(t p) n -> p t n", p=P)

            wk_sb = io.tile([P, CT, 256], f16, tag="wk", name="wk_sb")
            nc.sync.dma_start(
                wk_sb[:], wkT_d[:].rearrange("p (t m) -> p t m", t=CT))
            wq_sb = io.tile([P, CT, 256], f16, tag="wq", name="wq_sb")
            nc.sync.dma_start(
                wq_sb[:], wqT_d[:].rearrange("p (t m) -> p t m", t=CT))
            for t in range(CT):
                nc.sync.dma_start(xT_sb[:, t, 0:512], xT_ap[:, t, 0:512])
            wv_sb = io.tile([P, CT, 256], f16, tag="wv", name="wv_sb")
            nc.sync.dma_start(
                wv_sb[:], wvT_d[:].rearrange("p (t m) -> p t m", t=CT))
            pw_sb = io.tile([P, 2, C], f16, tag="pw", name="pw_sb")
            nc.sync.dma_start(
                pw_sb[:], pwT_d[:].rearrange("p (t m) -> p t m", t=2))
            for cc in range(1, QC):
                cs = slice(512 * cc, 512 * (cc + 1))
                for t in range(CT):
                    nc.sync.dma_start(xT_sb[:, t, cs], xT_ap[:, t, cs])

            # ---- SBUF persistents (packed: no zero padding) ----
            qT = []
            kT = []
            vv = []
            outT = []
            for p in range(2):
                qT.append(qk.tile([P, N], f16, tag=f"qT{p}", name=f"qT{p}"))
                kT.append(qk.tile([P, N], f16, tag=f"kT{p}", name=f"kT{p}"))
                vv.append(qk.tile([P, NT, 130], f16, tag=f"v{p}",
                                  name=f"v{p}"))
                outT.append(qk.tile([P, N], f16, tag=f"outT{p}",
                                    name=f"outT{p}"))

            # trigger the ACT exp table load during the DMA ramp
            scratch1 = io.tile([1, 2], f32, tag="scratch1", name="scratch1")
            nc.vector.memset(scratch1[:], 0.0)
            nc.scalar.activation(scratch1[0:1, 0:1], scratch1[0:1, 1:2], EXP)
            for p in range(2):
                # ones columns (fp16 1.0) at the head of each v block
                nc.vector.memset(vv[p][:, :, 0:1].bitcast(u16), 0x3C00)
                nc.vector.memset(vv[p][:, :, 65:66].bitcast(u16), 0x3C00)

            def emit_qk_chunk(p, w_sb, dst, ch):
                pc = slice(128 * p, 128 * (p + 1))
                cs = slice(512 * ch, 512 * (ch + 1))
                ps = ps_s.tile([P, 1024], f32, tag="s",
                               name=f"qkps_{p}_{ch}_{w_sb.tensor.name}")
                for t in range(CT):
                    nc.tensor.matmul(
                        ps[:, :512],
                        lhsT=w_sb[:, t, pc],
                        rhs=xT_sb[:, t, cs],
                        start=(t == 0), stop=(t == CT - 1))
                nc.vector.tensor_copy(dst[:, cs], ps[:, :512])

            def emit_v_tile(tt):
                psv = ps_s.tile([P, 1024], f32, tag="s", name=f"vps_{tt}")
                for t in range(CT):
                    nc.tensor.matmul(
                        psv[:, :256],
                        lhsT=xT_sb[:, t, 128 * tt:128 * (tt + 1)],
                        rhs=wv_sb[:, t, 0:256],
                        start=(t == 0), stop=(t == CT - 1))
                for p in range(2):
                    pv = psv[:, 128 * p:128 * (p + 1)].rearrange(
                        "p (two d) -> p two d", two=2)
                    dv = vv[p][:, tt, 0:130].rearrange(
                        "p (two d65) -> p two d65", two=2)[:, :, 1:65]
                    nc.vector.tensor_copy(dv, pv)

            def emit_y_block(tt):
                yps = ps_s.tile([P, 1024], f32, tag="s", name=f"yps_{tt}")
                for p in range(2):
                    nc.tensor.matmul(
                        yps[:, :512],
                        lhsT=outT[p][:, 128 * tt:128 * (tt + 1)],
                        rhs=pw_sb[:, p, :], start=(p == 0), stop=(p == 1))
                ys = yp.tile([P, C], f32, tag="y", name=f"ys_{tt}")
                nc.vector.tensor_copy(ys[:], yps[:, :512])
                nc.sync.dma_start(y_d[128 * tt:128 * (tt + 1), :], ys[:])

            fillers = []

            def pop_fillers(k):
                for _ in range(k):
                    if fillers:
                        fillers.pop(0)()

            def norm_head(p, hh, qc, o):
                qs = slice(512 * qc, 512 * (qc + 1))
                r = workp.tile([1, 512], f32, tag="r",
                               name=f"r_{p}_{hh}_{qc}")
                nc.vector.reciprocal_approx_fast(r[0:1, :], o[0:1, :])
                rb = workp.tile([65, 512], f32, tag="rb",
                                name=f"rb_{p}_{hh}_{qc}")
                nc.gpsimd.partition_broadcast(rb[:], r[0:1, :])
                st = workp.tile([65, 512], f16, tag="st",
                                name=f"st_{p}_{hh}_{qc}")
                nc.vector.tensor_mul(st[:], o[:], rb[:])
                nc.sync.dma_start(outT[p][64 * hh:64 * (hh + 1), qs],
                                  st[1:65, :])

            # ---- continuous block stream over all sections --------------
            # PE program order keeps a 2-block scores lookahead ahead of the
            # exp-gated attnv so ACT/DVE exps never wait on a scores matmul:
            # ... sP(b+1), aP(b-1), sP(b+2), aP(b), sP(b+3), ...
            sec_o = {}

            def emit_scores_exp(sec, p, qc, i, on_dve):
                qs = slice(512 * qc, 512 * (qc + 1))
                ks = slice(128 * i, 128 * (i + 1))
                s = ps_s.tile([P, 1024], f32, tag="s",
                              name=f"s_{p}_{qc}_{i}")
                nc.tensor.matmul(
                    s[:, 0:512], lhsT=kT[p][0:64, ks],
                    rhs=qT[p][0:64, qs], start=True, stop=True,
                    tile_position=(0, 0))
                nc.tensor.matmul(
                    s[:, 512:1024], lhsT=kT[p][64:128, ks],
                    rhs=qT[p][64:128, qs], start=True, stop=True,
                    tile_position=(64, 0))
                e = expp.tile([P, 1024], f16, tag="exp",
                              name=f"e_{p}_{qc}_{i}")
                if on_dve:
                    nc.vector._custom_dve(exp_op, out=e[:], in0=s[:],
                                          s0=EC3, s1=EC2, imm2=EC1)
                else:
                    nc.scalar.activation(e[:], s[:], EXP)
                return e

            def emit_attnv(sec, p, qc, i, e):
                if i == 0:
                    sec_o[sec] = (
                        ps_o.tile([65, 512], f32, tag="o",
                                  name=f"oA_{p}_{qc}"),
                        ps_o.tile([65, 512], f32, tag="o",
                                  name=f"oB_{p}_{qc}"),
                    )
                oA, oB = sec_o[sec]
                nc.tensor.matmul(
                    oA[:], lhsT=vv[p][:, i, 0:65], rhs=e[:, 0:512],
                    start=(i == 0), stop=(i == NT - 1))
                nc.tensor.matmul(
                    oB[:], lhsT=vv[p][:, i, 65:130], rhs=e[:, 512:1024],
                    start=(i == 0), stop=(i == NT - 1))
                if i == NT - 1:
                    norm_head(p, 0, qc, oA)
                    norm_head(p, 1, qc, oB)
                    del sec_o[sec]

            # critical prefix: what section (0, 0) touches first
            emit_qk_chunk(0, wk_sb, kT[0], 0)
            emit_qk_chunk(0, wk_sb, kT[0], 1)
            emit_qk_chunk(0, wq_sb, qT[0], 0)
            for tt in range(4):
                emit_v_tile(tt)

            def fqk(p, w_sb, dst, ch):
                fillers.append(lambda: emit_qk_chunk(p, w_sb, dst, ch))

            # the rest of phase A + phase C trickles in between blocks.
            # S0 pops (1/block): v tiles just ahead of attnv; k0 chunks 2-3
            # ahead of scores blocks 8/12; pair-1 k/q evicted before S1.
            fillers.append(lambda: emit_v_tile(4))
            fillers.append(lambda: emit_v_tile(5))
            fqk(0, wk_sb, kT[0], 2)
            fillers.append(lambda: emit_v_tile(6))
            fillers.append(lambda: emit_v_tile(7))
            fillers.append(lambda: emit_v_tile(8))
            fillers.append(lambda: emit_v_tile(9))
            fqk(0, wk_sb, kT[0], 3)
            fillers.append(lambda: emit_v_tile(10))
            fillers.append(lambda: emit_v_tile(11))
            fillers.append(lambda: emit_v_tile(12))
            fqk(1, wq_sb, qT[1], 0)
            fillers.append(lambda: emit_v_tile(13))
            fqk(1, wk_sb, kT[1], 0)
            fillers.append(lambda: emit_v_tile(14))
            fillers.append(lambda: emit_v_tile(15))
            # popped during S1:
            fqk(1, wk_sb, kT[1], 1)
            fqk(1, wk_sb, kT[1], 2)
            fqk(1, wk_sb, kT[1], 3)
            fqk(0, wq_sb, qT[0], 1)
            fqk(1, wq_sb, qT[1], 1)

            # sections: (pair, q-chunk), q-chunk-major so y blocks free early
            sections = [(p, qc) for qc in range(QC) for p in range(2)]
            # exp blocks handled by the DVE custom op (rest on ACT).
            # section starts stay on ACT so the previous section's norm isn't
            # queued behind a 1.2us DVE exp.
            DVE_BLOCKS = {
                0: (), 1: (3, 7, 11),
                2: (3, 6, 9, 12, 14), 3: (3, 6, 9, 12, 14),
                4: (3, 6, 9, 12, 14), 5: (3, 6, 9, 12, 14),
                6: (3, 6, 9, 12, 14), 7: (3, 6, 9, 12, 14),
            }
            blocks = [(idx, p, qc, i)
                      for idx, (p, qc) in enumerate(sections)
                      for i in range(NT)]
            pend = []
            for b, (idx, p, qc, i) in enumerate(blocks):
                e = emit_scores_exp(idx, p, qc, i, i in DVE_BLOCKS[idx])
                pend.append((idx, p, qc, i, e))
                if b >= 2:
                    emit_attnv(*pend.pop(0))
                if idx == 2 and i == 0:
                    # y blocks 0-3 (tokens 0-511): sections 0-1 norms land
                    # ~2 blocks into section 2; pad so PE never waits on them
                    fqk(0, wq_sb, qT[0], 2)
                    fqk(1, wq_sb, qT[1], 2)
                    for _ in range(3):
                        fillers.append(lambda: None)
                    for tt in range(4):
                        fillers.append(lambda tt=tt: emit_y_block(tt))
                elif idx == 4 and i == 0:
                    fqk(0, wq_sb, qT[0], 3)
                    fqk(1, wq_sb, qT[1], 3)
                    for _ in range(3):
                        fillers.append(lambda: None)
                    for tt in range(4, 8):
                        fillers.append(lambda tt=tt: emit_y_block(tt))
                elif idx == 6 and i == 0:
                    for _ in range(5):
                        fillers.append(lambda: None)
                    for tt in range(8, 12):
                        fillers.append(lambda tt=tt: emit_y_block(tt))
                pop_fillers(1)
            while pend:
                emit_attnv(*pend.pop(0))
            pop_fillers(len(fillers))

            # ---- tail: last y blocks ----
            for tt in range(12, NT):
                emit_y_block(tt)

    nc.finalize()
    return nc


def _get_nc():
    if "nc" not in _cache:
        _cache["nc"] = _build()
    return _cache["nc"]


def _pack(wt, groups):
    # [G*128, M] row-major -> [128, G*M]: partition p holds the concat over
    # groups of row (g*128 + p), so the DMA reads one contiguous run per p
    g128, m = wt.shape
    assert g128 == groups * 128
    return np.ascontiguousarray(
        wt.reshape(groups, 128, m).transpose(1, 0, 2).reshape(128, groups * m))


def _make_in_maps(x, q_w, kv_w, proj_w):
    x = np.asarray(x, dtype=np.float32)
    q_w = np.asarray(q_w, dtype=np.float32)
    kv_w = np.asarray(kv_w, dtype=np.float32)
    proj_w = np.asarray(proj_w, dtype=np.float32)
    f16 = np.float16
    in_maps = []
    for core in range(NCORES):
        b, g = core // 2, core % 2
        hs = slice(g * 256, (g + 1) * 256)
        in_maps.append({
            "xT": np.ascontiguousarray(x[b].T.astype(f16)),
            "wqT": _pack((q_w[hs, :] * np.float32(SCALE)).T.astype(f16), CT),
            "wkT": _pack(kv_w[hs, :].T.astype(f16), CT),
            "wvT": _pack(
                kv_w[C + g * 256:C + (g + 1) * 256, :].T.astype(f16), CT),
            "pwT": _pack(proj_w[:, hs].T.astype(f16), 2),
        })
    return in_maps


def kernel(x, q_w, kv_w, proj_w, proj_b, H=None, W=None, _trace=False):
    from concourse.bass_utils import run_bass_kernel_spmd

    nc = _get_nc()
    in_maps = _make_in_maps(x, q_w, kv_w, proj_w)
    res = run_bass_kernel_spmd(nc, in_maps, core_ids=list(range(NCORES)),
                               trace=_trace)
    proj_b = np.asarray(proj_b, dtype=np.float32)
    out = np.empty((B, N, C), dtype=np.float32)
    for b in range(B):
        out[b] = res.results[2 * b]["y"] + res.results[2 * b + 1]["y"] + proj_b
    if _trace:
        return out, res
    return out


# revision 3
# speedup vs baseline: 1.2886x; 1.0431x over previous
"""Trainium2 Bass kernel for multi-head attention (B=4, N=2048, C=512, 8 heads).

Sharding: 8 cores = (batch b = core//2) x (head-group g = core%2, 4 heads each).

v2 pipeline (per core, 4 heads as 2 pairs):
  - q/k stored PACKED fp16: qT[p]/kT[p] [128 = headA d0-63 | headB d0-63, N].
    No zero padding anywhere: scores for the two heads of a pair run as two
    CONCURRENT row-tiled K=64 matmuls (tile_position (0,0) / (64,0)) writing
    the two halves of one [128, 1024] PSUM tile -> the pair's score block
    costs one matmul span (~220ns) instead of two.
  - one exp per block over the paired [128, 1024] PSUM tile. Exp work is
    split between ACT (nc.scalar.activation Exp) and a custom 8-stage DVE
    op (EXP_POLY3SQ2_ANT: cubic in x then two squarings ~ exp(x), max rel
    err 1.9e-3 on |logit| <= 2.05) so the two engines share the softmax.
  - v as [N, (1+64) per head] tiles; leading ones column emits the softmax
    denominator into PSUM row 0 of each head's [65, 512] accumulator.
  - sections = (pair p, q-chunk of 512) x 16 k-blocks; attnv trails scores
    by one block (in-order PE must never wait on a just-issued exp).
  - phase-A work (qkv projections, v tiles) and phase-C y-blocks trickle in
    as per-block fillers so ACT/DVE start early and PE never idles long
    (keeps the HAM clock gate at 2.4 GHz).
  - normalization off the PE: DVE fast-reciprocal, GpSimd partition
    broadcast, DVE multiply, DMA partition-shift into outT.
  - output projection on-device; host sums the two half-head partials.
"""

import sys

sys.path.insert(0, "/opt/trn_rl_repo")

import numpy as np

B, N, C = 4, 2048, 512
H, D = 8, 64
SCALE = float(D) ** -0.5  # 0.125, exact
P = 128
CT = C // P   # 4 contraction tiles over channels
NT = N // P   # 16 token blocks
QC = 4        # q chunks of 512
NCORES = 8

# custom DVE exp: p(x) = ((c3*x + c2)*x + c1)*x + 1, exp(x) ~ p(x)^4
EC3, EC2, EC1 = 0.0025544826062447396, 0.03181193776331223, 0.2502295107773785

_cache = {}


def _register_exp_op():
    import concourse.dve_ops as dve_ops
    from concourse.dve_ops import DveOp
    from concourse.dve_spec import C0 as _C0
    from concourse.dve_spec import C1 as _C1
    from concourse.dve_spec import C2 as _C2
    from concourse.dve_spec import One, Spec, Src0, lower, sq
    from concourse.dve_uop import DveOpSpec

    name = "EXP_POLY3SQ2_ANT"
    for op in dve_ops.OPS:
        if op.name == name:
            return op

    body = sq(sq(((Src0 * _C0 + _C1) * Src0 + _C2) * Src0 + One))

    def ref(in0, in1, c0, c1, c2):
        p = ((in0 * c0 + c1) * in0 + c2) * in0 + 1.0
        p = p * p
        return (p * p).astype(np.float32)

    spec = Spec(body=body, reference=ref)
    row = dve_ops._CUSTOM_DVE_ROW_BASE + len(dve_ops.OPS)
    assert row < 0x20
    dve_ops._SUB_OPCODE_FOR_NAME[name] = row
    shas = {}
    for ver in ("v3", "v4"):
        d = DveOpSpec(name=name, opcode=row, uops=lower(spec, ver=ver),
                      rd1_en=False)
        shas[ver] = d.sha(ver)
    op = DveOp(name, spec, subdim=False, uops_sha=shas)
    dve_ops.OPS.append(op)
    dve_ops.CUSTOM_DVE_SPECS[name] = spec
    return op


def _build():
    import concourse.bacc as bacc
    import concourse.tile as tile
    from concourse import mybir

    f32 = mybir.dt.float32
    f16 = mybir.dt.float16
    u16 = mybir.dt.uint16
    EXP = mybir.ActivationFunctionType.Exp

    exp_op = _register_exp_op()

    nc = bacc.Bacc("TRN2", target_bir_lowering=False, debug=False,
                   num_devices=NCORES)

    xT_d = nc.dram_tensor("xT", [C, N], f16, kind="ExternalInput")
    wqT_d = nc.dram_tensor("wqT", [P, CT * 256], f16, kind="ExternalInput")
    wkT_d = nc.dram_tensor("wkT", [P, CT * 256], f16, kind="ExternalInput")
    wvT_d = nc.dram_tensor("wvT", [P, CT * 256], f16, kind="ExternalInput")
    pwT_d = nc.dram_tensor("pwT", [P, 2 * C], f16, kind="ExternalInput")
    y_d = nc.dram_tensor("y", [N, C], f32, kind="ExternalOutput")

    with tile.TileContext(nc) as tc:
        with (
            tc.tile_pool(name="io", bufs=1) as io,
            tc.tile_pool(name="qk", bufs=1) as qk,
            tc.tile_pool(name="expp", bufs=7) as expp,
            tc.tile_pool(name="workp", bufs=4) as workp,
            tc.tile_pool(name="yp", bufs=4) as yp,
            tc.tile_pool(name="ps_s", bufs=3, space="PSUM") as ps_s,
            tc.tile_pool(name="ps_o", bufs=2, space="PSUM") as ps_o,
        ):
            # ---- input loads (fine-grained: first matmul starts early) ----
            xT_sb = io.tile([P, CT, N], f16, tag="xT", name="xT_sb")
            xT_ap = xT_d[:].rearrange("(t p) n -> p t n", p=P)

            for t in range(CT):
                nc.sync.dma_start(xT_sb[:, t, 0:512], xT_ap[:, t, 0:512])
            wk_sb = io.tile([P, CT, 256], f16, tag="wk", name="wk_sb")
            nc.sync.dma_start(
                wk_sb[:], wkT_d[:].rearrange("p (t m) -> p t m", t=CT))
            wq_sb = io.tile([P, CT, 256], f16, tag="wq", name="wq_sb")
            nc.sync.dma_start(
                wq_sb[:], wqT_d[:].rearrange("p (t m) -> p t m", t=CT))
            wv_sb = io.tile([P, CT, 256], f16, tag="wv", name="wv_sb")
            nc.sync.dma_start(
                wv_sb[:], wvT_d[:].rearrange("p (t m) -> p t m", t=CT))
            pw_sb = io.tile([P, 2, C], f16, tag="pw", name="pw_sb")
            nc.sync.dma_start(
                pw_sb[:], pwT_d[:].rearrange("p (t m) -> p t m", t=2))
            for cc in range(1, QC):
                cs = slice(512 * cc, 512 * (cc + 1))
                for t in range(CT):
                    nc.sync.dma_start(xT_sb[:, t, cs], xT_ap[:, t, cs])

            # ---- SBUF persistents (packed: no zero padding) ----
            qT = []
            kT = []
            vv = []
            outT = []
            for p in range(2):
                qT.append(qk.tile([P, N], f16, tag=f"qT{p}", name=f"qT{p}"))
                kT.append(qk.tile([P, N], f16, tag=f"kT{p}", name=f"kT{p}"))
                vv.append(qk.tile([P, NT, 130], f16, tag=f"v{p}",
                                  name=f"v{p}"))
                outT.append(qk.tile([P, N], f16, tag=f"outT{p}",
                                    name=f"outT{p}"))

            # trigger the ACT exp table load during the DMA ramp
            scratch1 = io.tile([1, 2], f32, tag="scratch1", name="scratch1")
            nc.vector.memset(scratch1[:], 0.0)
            nc.scalar.activation(scratch1[0:1, 0:1], scratch1[0:1, 1:2], EXP)
            for p in range(2):
                # ones columns (fp16 1.0) at the head of each v block
                nc.vector.memset(vv[p][:, :, 0:1].bitcast(u16), 0x3C00)
                nc.vector.memset(vv[p][:, :, 65:66].bitcast(u16), 0x3C00)

            def emit_qk_chunk(p, w_sb, dst, ch):
                pc = slice(128 * p, 128 * (p + 1))
                cs = slice(512 * ch, 512 * (ch + 1))
                ps = ps_s.tile([P, 1024], f32, tag="s",
                               name=f"qkps_{p}_{ch}_{w_sb.tensor.name}")
                for t in range(CT):
                    nc.tensor.matmul(
                        ps[:, :512],
                        lhsT=w_sb[:, t, pc],
                        rhs=xT_sb[:, t, cs],
                        start=(t == 0), stop=(t == CT - 1))
                nc.vector.tensor_copy(dst[:, cs], ps[:, :512])

            def emit_v_tile(tt):
                psv = ps_s.tile([P, 1024], f32, tag="s", name=f"vps_{tt}")
                for t in range(CT):
                    nc.tensor.matmul(
                        psv[:, :256],
                        lhsT=xT_sb[:, t, 128 * tt:128 * (tt + 1)],
                        rhs=wv_sb[:, t, 0:256],
                        start=(t == 0), stop=(t == CT - 1))
                for p in range(2):
                    pv = psv[:, 128 * p:128 * (p + 1)].rearrange(
                        "p (two d) -> p two d", two=2)
                    dv = vv[p][:, tt, 0:130].rearrange(
                        "p (two d65) -> p two d65", two=2)[:, :, 1:65]
                    nc.vector.tensor_copy(dv, pv)

            def emit_y_block(tt):
                yps = ps_s.tile([P, 1024], f32, tag="s", name=f"yps_{tt}")
                for p in range(2):
                    nc.tensor.matmul(
                        yps[:, :512],
                        lhsT=outT[p][:, 128 * tt:128 * (tt + 1)],
                        rhs=pw_sb[:, p, :], start=(p == 0), stop=(p == 1))
                ys = yp.tile([P, C], f32, tag="y", name=f"ys_{tt}")
                nc.vector.tensor_copy(ys[:], yps[:, :512])
                nc.sync.dma_start(y_d[128 * tt:128 * (tt + 1), :], ys[:])

            fillers = []

            def pop_fillers(k):
                for _ in range(k):
                    if fillers:
                        fillers.pop(0)()

            def norm_head(p, hh, qc, o):
                qs = slice(512 * qc, 512 * (qc + 1))
                r = workp.tile([1, 512], f32, tag="r",
                               name=f"r_{p}_{hh}_{qc}")
                nc.vector.reciprocal_approx_fast(r[0:1, :], o[0:1, :])
                rb = workp.tile([65, 512], f32, tag="rb",
                                name=f"rb_{p}_{hh}_{qc}")
                nc.gpsimd.partition_broadcast(rb[:], r[0:1, :])
                st = workp.tile([65, 512], f16, tag="st",
                                name=f"st_{p}_{hh}_{qc}")
                nc.vector.tensor_mul(st[:], o[:], rb[:])
                nc.sync.dma_start(outT[p][64 * hh:64 * (hh + 1), qs],
                                  st[1:65, :])

            # ---- continuous block stream over all sections --------------
            # PE program order keeps a 2-block scores lookahead ahead of the
            # exp-gated attnv so ACT/DVE exps never wait on a scores matmul:
            # ... sP(b+1), aP(b-1), sP(b+2), aP(b), sP(b+3), ...
            sec_o = {}

            def emit_scores_exp(sec, p, qc, i, on_dve):
                qs = slice(512 * qc, 512 * (qc + 1))
                ks = slice(128 * i, 128 * (i + 1))
                s = ps_s.tile([P, 1024], f32, tag="s",
                              name=f"s_{p}_{qc}_{i}")
                nc.tensor.matmul(
                    s[:, 0:512], lhsT=kT[p][0:64, ks],
                    rhs=qT[p][0:64, qs], start=True, stop=True,
                    tile_position=(0, 0))
                nc.tensor.matmul(
                    s[:, 512:1024], lhsT=kT[p][64:128, ks],
                    rhs=qT[p][64:128, qs], start=True, stop=True,
                    tile_position=(64, 0))
                e = expp.tile([P, 1024], f16, tag="exp",
                              name=f"e_{p}_{qc}_{i}")
                if on_dve:
                    nc.vector._custom_dve(exp_op, out=e[:], in0=s[:],
                                          s0=EC3, s1=EC2, imm2=EC1)
                else:
                    nc.scalar.activation(e[:], s[:], EXP)
                return e

            def emit_attnv(sec, p, qc, i, e):
                if i == 0:
                    sec_o[sec] = (
                        ps_o.tile([65, 512], f32, tag="o",
                                  name=f"oA_{p}_{qc}"),
                        ps_o.tile([65, 512], f32, tag="o",
                                  name=f"oB_{p}_{qc}"),
                    )
                oA, oB = sec_o[sec]
                nc.tensor.matmul(
                    oA[:], lhsT=vv[p][:, i, 0:65], rhs=e[:, 0:512],
                    start=(i == 0), stop=(i == NT - 1))
                nc.tensor.matmul(
                    oB[:], lhsT=vv[p][:, i, 65:130], rhs=e[:, 512:1024],
                    start=(i == 0), stop=(i == NT - 1))
                if i == NT - 1:
                    norm_head(p, 0, qc, oA)
                    norm_head(p, 1, qc, oB)
                    del sec_o[sec]

            # critical prefix: the bare minimum before the scores stream
            emit_qk_chunk(0, wk_sb, kT[0], 0)
            emit_qk_chunk(0, wq_sb, qT[0], 0)
            emit_v_tile(0)
            emit_v_tile(1)

            def fqk(p, w_sb, dst, ch):
                fillers.append(lambda: emit_qk_chunk(p, w_sb, dst, ch))

            def fv(tt):
                fillers.append(lambda: emit_v_tile(tt))

            # the rest of phase A + phase C trickles in between blocks.
            # S0 pops 1/block (2 at i=4/8/12): v tiles ahead of the lag-4
            # attnv; k0 chunks ahead of scores blocks 4/8/12; pair-1 k/q
            # evicted before S1 starts.
            fqk(0, wk_sb, kT[0], 1)
            fv(2)
            fv(3)
            fv(4)
            fqk(0, wk_sb, kT[0], 2)
            fv(5)
            fv(6)
            fv(7)
            fv(8)
            fqk(0, wk_sb, kT[0], 3)
            fv(9)
            fv(10)
            fv(11)
            fv(12)
            fqk(1, wq_sb, qT[1], 0)
            fqk(1, wk_sb, kT[1], 0)
            fv(13)
            fv(14)
            fv(15)
            # popped during S1:
            fqk(1, wk_sb, kT[1], 1)
            fqk(1, wk_sb, kT[1], 2)
            fqk(1, wk_sb, kT[1], 3)
            fqk(0, wq_sb, qT[0], 1)
            fqk(1, wq_sb, qT[1], 1)

            # sections: (pair, q-chunk), q-chunk-major so y blocks free early
            sections = [(p, qc) for qc in range(QC) for p in range(2)]
            # exp blocks handled by the DVE custom op (rest on ACT).
            # section starts stay on ACT so the previous section's norm isn't
            # queued behind a 1.2us DVE exp.
            DVE_BLOCKS = {
                0: (6, 11), 1: (2, 5, 8, 11, 14),
                2: (2, 5, 8, 11, 14), 3: (2, 5, 8, 11, 14),
                4: (2, 5, 8, 11, 14), 5: (2, 5, 8, 11, 14),
                6: (2, 5, 8, 11, 14), 7: (2, 5, 8, 11, 14),
            }
            blocks = [(idx, p, qc, i)
                      for idx, (p, qc) in enumerate(sections)
                      for i in range(NT)]
            pend = []
            for b, (idx, p, qc, i) in enumerate(blocks):
                e = emit_scores_exp(idx, p, qc, i, i in DVE_BLOCKS[idx])
                pend.append((idx, p, qc, i, e))
                if b >= 4:
                    emit_attnv(*pend.pop(0))
                if idx == 2 and i == 0:
                    # y blocks 0-3 (tokens 0-511) ready once sections 0-1
                    # norms land (~2 blocks into section 2)
                    fqk(0, wq_sb, qT[0], 2)
                    fqk(1, wq_sb, qT[1], 2)
                    for tt in range(4):
                        fillers.append(lambda tt=tt: emit_y_block(tt))
                elif idx == 4 and i == 0:
                    fqk(0, wq_sb, qT[0], 3)
                    fqk(1, wq_sb, qT[1], 3)
                    for tt in range(4, 8):
                        fillers.append(lambda tt=tt: emit_y_block(tt))
                elif idx == 6 and i == 0:
                    for tt in range(8, 12):
                        fillers.append(lambda tt=tt: emit_y_block(tt))
                # early sections drain their many fillers every block; later
                # sections space fillers out so PSUM slot reuse never stalls
                # the scores stream, and keep boundaries clear
                if idx == 0:
                    pop_fillers(2 if i in (4, 8, 12) else 1)
                elif idx == 1:
                    pop_fillers(1)
                elif idx in (2, 4) and i in (2, 4, 6, 8, 10, 12):
                    pop_fillers(1)
                elif idx == 6 and i in (4, 6, 8, 10):
                    pop_fillers(1)
            while pend:
                emit_attnv(*pend.pop(0))
            pop_fillers(len(fillers))

            # ---- tail: last y blocks ----
            for tt in range(12, NT):
                emit_y_block(tt)

    nc.finalize()
    return nc


def _get_nc():
    if "nc" not in _cache:
        _cache["nc"] = _build()
    return _cache["nc"]


def _pack(wt, groups):
    # [G*128, M] row-major -> [128, G*M]: partition p holds the concat over
    # groups of row (g*128 + p), so the DMA reads one contiguous run per p
    g128, m = wt.shape
    assert g128 == groups * 128
    return np.ascontiguousarray(
        wt.reshape(groups, 128, m).transpose(1, 0, 2).reshape(128, groups * m))


def _make_in_maps(x, q_w, kv_w, proj_w):
    x = np.asarray(x, dtype=np.float32)
    q_w = np.asarray(q_w, dtype=np.float32)
    kv_w = np.asarray(kv_w, dtype=np.float32)
    proj_w = np.asarray(proj_w, dtype=np.float32)
    f16 = np.float16
    in_maps = []
    for core in range(NCORES):
        b, g = core // 2, core % 2
        hs = slice(g * 256, (g + 1) * 256)
        in_maps.append({
            "xT": np.ascontiguousarray(x[b].T.astype(f16)),
            "wqT": _pack((q_w[hs, :] * np.float32(SCALE)).T.astype(f16), CT),
            "wkT": _pack(kv_w[hs, :].T.astype(f16), CT),
            "wvT": _pack(
                kv_w[C + g * 256:C + (g + 1) * 256, :].T.astype(f16), CT),
            "pwT": _pack(proj_w[:, hs].T.astype(f16), 2),
        })
    return in_maps


def kernel(x, q_w, kv_w, proj_w, proj_b, H=None, W=None, _trace=False):
    from concourse.bass_utils import run_bass_kernel_spmd

    nc = _get_nc()
    in_maps = _make_in_maps(x, q_w, kv_w, proj_w)
    res = run_bass_kernel_spmd(nc, in_maps, core_ids=list(range(NCORES)),
                               trace=_trace)
    proj_b = np.asarray(proj_b, dtype=np.float32)
    out = np.empty((B, N, C), dtype=np.float32)
    for b in range(B):
        out[b] = res.results[2 * b]["y"] + res.results[2 * b + 1]["y"] + proj_b
    if _trace:
        return out, res
    return out
